# revision 32
# baseline (speedup 1.0000x reference)
"""DigitCaps dynamic-routing kernel for Trainium2 (8 NeuronCores, batch-sharded).

Full-input contract: kernel(x, y, W) -> (256, 10, 16) fp32.
  x: (256, 1152, 8) fp32, y: (256, 10) fp32 (unused by the reference), W: (1, 10, 1152, 16, 8) fp32.

Strategy (per core, 32 samples):
  - u_hat = einsum('oidk,bik->boid') computed on the PE array as 288 matmuls with a
    block-diagonal x operand: contraction dim = (i_local16, k8) = 128, output
    partitions = (i_local16, b8) = 128, moving dim = (o,d) = 160 per i-group.
  - u_hat kept resident in SBUF as fp16 [(il,b)=128, g=72, o=10, d=16]; the routing
    loop never touches HBM.
  - Routing iteration: t = c*u (DVE fp16 2x) -> s = col-sum via ones-block matmul
    (PE, PSUM accumulate over i-groups) -> broadcast s across partitions (DMA)
    -> p = sum_d u*s via DVE pairwise halving tree -> b-logit update from the
    closed form  b += f(sq)*(p - usq), sq = |s|^2 - 2p + usq, f = squash scale.
  - Final pass: s3 accumulated fp32 in PSUM, squash applied on fp32.
"""

import sys
from contextlib import ExitStack

sys.path.insert(0, "/opt/trn_rl_repo")

import functools

import numpy as np

from concourse import bacc, mybir, tile
from concourse import hw_specs as _hw_specs
from concourse.bass_utils import run_bass_kernel_spmd

# All ACT functions this kernel uses (Exp, Ln, Square, Copy, Identity) live
# together in the "natural_log_exp_and_others" table set, but the default
# placement pass maps Exp->set 0 and Ln->set 5, thrashing ~2.7us table loads
# between them on every routing iteration. Restrict those functions to the
# combined set (set ids stay positional, so other sets keep their ids).
_orig_get_activation_tables = _hw_specs.get_activation_tables


@functools.cache
def _patched_activation_tables(module_arch):
    tables = dict(_orig_get_activation_tables(module_arch))
    shared = None
    for name, funcs in tables.items():
        if name == "natural_log_exp_and_others":
            shared = funcs
    if shared is None:
        return tables
    strip = {
        f
        for f in (
            getattr(mybir.ActivationFunctionType, n, None)
            for n in ("Exp", "Ln", "Square", "Copy", "Identity")
        )
        if f is not None and f in shared
    }
    return {
        name: (funcs if name == "natural_log_exp_and_others" else funcs - strip)
        for name, funcs in tables.items()
    }


_hw_specs.get_activation_tables = _patched_activation_tables
bacc.get_activation_tables = _patched_activation_tables

F16 = mybir.dt.float16
F32 = mybir.dt.float32

N_CORES = 8
BL = 32          # batch per core
NG = 72          # i-groups (1152 / 16)
IL = 16          # i's per group
KD = 8           # in_dim
O = 10           # out_caps
D = 16           # out_dim
OD = O * D       # 160
NBG = 4          # sample-groups of 8 per core
GB = 8           # samples per group
EPS = 1e-8

_NC = None


def _build_module(repeat=1):
    nc = bacc.Bacc("TRN2", target_bir_lowering=False, debug=False)

    xd_d = nc.dram_tensor("xd", [128, NBG, NG, 128], F16, kind="ExternalInput")
    w_d = nc.dram_tensor("wr", [128, NG, OD], F16, kind="ExternalInput")
    ones_d = nc.dram_tensor("ones8", [128, GB], F16, kind="ExternalInput")
    e8_d = nc.dram_tensor("e8", [GB, 128], F16, kind="ExternalInput")
    out_d = nc.dram_tensor("out", [BL, O, D], F32, kind="ExternalOutput")

    with tile.TileContext(nc) as tc, ExitStack() as ctx:
        consts = ctx.enter_context(tc.tile_pool(name="consts", bufs=1))
        upool = ctx.enter_context(tc.tile_pool(name="u", bufs=2))
        tpool = ctx.enter_context(tc.tile_pool(name="t", bufs=2))
        spool = ctx.enter_context(tc.tile_pool(name="smalls", bufs=2))
        scr = ctx.enter_context(tc.tile_pool(name="scr", bufs=2))
        scr1 = ctx.enter_context(tc.tile_pool(name="scr1", bufs=2))
        lhsp = ctx.enter_context(tc.tile_pool(name="lhsp", bufs=2))
        psum = ctx.enter_context(tc.tile_pool(name="psum", bufs=3, space="PSUM"))
        psum_s = ctx.enter_context(tc.tile_pool(name="psum_s", bufs=1, space="PSUM"))
        psum_bc = ctx.enter_context(tc.tile_pool(name="psum_bc", bufs=2, space="PSUM"))

        w_t = consts.tile([128, NG, OD], F16)
        nc.sync.dma_start(w_t[:], w_d[:, :, :])
        ones_t = consts.tile([128, GB], F16)
        nc.sync.dma_start(ones_t[:], ones_d[:, :])
        e8_t = consts.tile([GB, 128], F16)
        nc.sync.dma_start(e8_t[:], e8_d[:, :])

        for bg in [b for _ in range(repeat) for b in range(NBG)]:
            # ---------------- phase 1: u_hat for this sample group ----------------
            # u is stored (d, g, o)-ordered so broadcasts in the routing loop
            # stay off the innermost axis (keeps DVE 2x_1P fp16 mode).
            u_t = upool.tile([128, D, NG, O], F16, tag="u")
            for quarter in range(4):
                g0 = quarter * 18
                lhs_t = lhsp.tile([128, 18, 128], F16, tag="lhs")
                nc.sync.dma_start(lhs_t[:], xd_d[:, bg, g0 : g0 + 18, :])
                for m in range(6):  # triples of groups
                    pt = psum.tile([128, 3, O, D], F32, tag="pp")
                    for j in range(3):
                        g = g0 + m * 3 + j
                        nc.tensor.matmul(
                            pt[:, j],
                            lhsT=lhs_t[:, g - g0, :],
                            rhs=w_t[:, g, :],
                            start=True,
                            stop=True,
                        )
                    # transposing copy psum (g,o,d) -> u (d,g,o)
                    dst = u_t[:, :, g0 + m * 3 : g0 + m * 3 + 3, :]
                    nc.scalar.copy(dst.rearrange("p d g o -> p g o d"), pt[:])

            t_t = tpool.tile([128, D, NG, O], F16, tag="t")
            a_t = scr.tile([128, 8, NG, O], F16, tag="a")
            b4_t = scr1.tile([128, 4, NG, O], F16, tag="b4")
            c2_t = scr1.tile([128, 2, NG, O], F16, tag="c2")
            sb2_t = scr1.tile([128, D, O], F16, tag="sb2")

            p_t = spool.tile([128, NG, O], F16, tag="p")
            usq_t = spool.tile([128, NG, O], F16, tag="usq")
            blog_t = spool.tile([128, NG, O], F32, tag="blog")
            sq_t = spool.tile([128, NG, O], F16, tag="sq")
            g_t = spool.tile([128, NG, O], F16, tag="g")
            tm_t = spool.tile([128, NG, O], F16, tag="tm")
            tn_t = spool.tile([128, NG, O], F16, tag="tn")
            e_t = spool.tile([128, NG, O], F16, tag="e")
            c_t = spool.tile([128, NG, O], F16, tag="c")
            sig_t = spool.tile([128, NG], F32, tag="sig")
            sb_t = spool.tile([128, D, O], F16, tag="sb")
            ssq_t = spool.tile([128, O], F16, tag="ssq")
            s2a_t = spool.tile([128, 4, O], F16, tag="s2a")
            s8f_t = spool.tile([8, D, O], F32, tag="s8f")
            s8h_t = spool.tile([8, D, O], F16, tag="s8h")
            ssq3_t = spool.tile([8, O], F32, tag="ssq3")
            sq3a_t = spool.tile([8, 8, O], F32, tag="sq3a")
            f3a_t = spool.tile([8, O], F32, tag="f3a")
            f3b_t = spool.tile([8, O], F32, tag="f3b")
            v_t = spool.tile([8, D, O], F32, tag="v")

            def halving_tree(src, dst, last_fp32=False):
                # src: [128, 16, NG, O] fp16 -> dst: [128, NG, O] (sum over dim 1)
                nc.vector.tensor_add(a_t[:], src[:, 0:8], src[:, 8:16])
                nc.vector.tensor_add(b4_t[:], a_t[:, 0:4], a_t[:, 4:8])
                nc.vector.tensor_add(c2_t[:], b4_t[:, 0:2], b4_t[:, 2:4])
                nc.vector.tensor_add(dst[:, None, :, :], c2_t[:, 0:1], c2_t[:, 1:2])

            # usq = sum_d u^2 (uses t_t as scratch for the squares)
            nc.scalar.square(t_t[:], u_t[:])
            halving_tree(t_t, usq_t)

            for it in range(3):
                # ---- t = c * u ----
                if it == 0:
                    pass  # c is uniform 0.1: col-sum u directly, fold 0.1 into s
                else:
                    # softmax over o of b-logits (logits are tiny; skip max-sub)
                    nc.scalar.activation(
                        e_t[:], blog_t[:], mybir.ActivationFunctionType.Exp
                    )
                    nc.vector.tensor_reduce(
                        sig_t[:], e_t[:], axis=mybir.AxisListType.X,
                        op=mybir.AluOpType.add,
                    )
                    nc.vector.reciprocal_approx_fast(sig_t[:], sig_t[:])
                    nc.vector.tensor_mul(
                        c_t[:], e_t[:], sig_t[:, :, None].to_broadcast((128, NG, O))
                    )
                    nc.vector.tensor_mul(
                        t_t[:], u_t[:],
                        c_t[:, None, :, :].to_broadcast((128, D, NG, O)),
                    )

                # ---- s = sum_i t  (ones-block matmul, 2 PSUM accumulators) ----
                src_t = u_t if it == 0 else t_t
                psA = psum_s.tile([8, D, 3, O], F32, tag="psA")
                psB = psum_s.tile([8, D, 3, O], F32, tag="psB")
                for m in range(24):
                    dst = psA if m % 2 == 0 else psB
                    nc.tensor.matmul(
                        dst[:],
                        lhsT=ones_t[:],
                        rhs=src_t[:, :, 3 * m : 3 * m + 3, :],
                        start=(m < 2),
                        stop=(m >= 22),
                    )
                nc.vector.tensor_copy(s8f_t[:, :, None, :], psA[:, :, 0:1])
                nc.vector.tensor_add(s8f_t[:, :, None, :], s8f_t[:, :, None, :], psA[:, :, 1:2])
                nc.vector.tensor_add(s8f_t[:, :, None, :], s8f_t[:, :, None, :], psA[:, :, 2:3])
                nc.vector.tensor_add(s8f_t[:, :, None, :], s8f_t[:, :, None, :], psB[:, :, 0:1])
                nc.vector.tensor_add(s8f_t[:, :, None, :], s8f_t[:, :, None, :], psB[:, :, 1:2])
                nc.vector.tensor_add(s8f_t[:, :, None, :], s8f_t[:, :, None, :], psB[:, :, 2:3])
                if it == 0:
                    nc.vector.tensor_scalar_mul(s8f_t[:], s8f_t[:], 0.1)

                if it == 2:
                    # ---- final squash(s) -> output (fp32) ----
                    nc.scalar.square(v_t[:], s8f_t[:])
                    nc.vector.tensor_add(sq3a_t[:], v_t[:, 0:8], v_t[:, 8:16])
                    nc.vector.tensor_add(sq3a_t[:, 0:4], sq3a_t[:, 0:4], sq3a_t[:, 4:8])
                    nc.vector.tensor_add(sq3a_t[:, 0:2], sq3a_t[:, 0:2], sq3a_t[:, 2:4])
                    nc.vector.tensor_add(
                        ssq3_t[:, None, :], sq3a_t[:, 0:1], sq3a_t[:, 1:2]
                    )
                    nc.scalar.add(f3a_t[:], ssq3_t[:], 1.0)
                    nc.scalar.activation(
                        f3b_t[:], ssq3_t[:], mybir.ActivationFunctionType.Ln
                    )
                    nc.scalar.activation(
                        f3b_t[:], f3b_t[:], mybir.ActivationFunctionType.Exp, scale=0.5
                    )
                    nc.vector.scalar_tensor_tensor(
                        f3a_t[:], f3b_t[:], EPS, f3a_t[:],
                        op0=mybir.AluOpType.add, op1=mybir.AluOpType.mult,
                    )
                    nc.vector.reciprocal(f3a_t[:], f3a_t[:])
                    nc.vector.tensor_mul(f3a_t[:], f3a_t[:], ssq3_t[:])
                    nc.vector.tensor_mul(
                        v_t[:], s8f_t[:], f3a_t[:, None, :].to_broadcast((8, D, O))
                    )
                    vo_t = spool.tile([8, O, D], F32, tag="vo")
                    nc.vector.tensor_copy(
                        vo_t[:].rearrange("b o d -> b d o"), v_t[:]
                    )
                    nc.sync.dma_start(out_d[bg * 8 : bg * 8 + 8], vo_t[:])
                    continue

                # ---- broadcast s across partitions (delta-matmul) ----
                nc.vector.tensor_copy(s8h_t[:], s8f_t[:])
                ps_bc = psum_bc.tile([128, D, O], F32, tag="bc")
                nc.tensor.matmul(
                    ps_bc[:], lhsT=e8_t[:], rhs=s8h_t[:], start=True, stop=True
                )
                nc.vector.tensor_copy(sb_t[:], ps_bc[:])

                # ssq = sum_d s^2 (tiny halving tree over d)
                nc.vector.tensor_mul(sb2_t[:], sb_t[:], sb_t[:])
                nc.vector.tensor_add(s2a_t[:, 0:4], sb2_t[:, 0:4], sb2_t[:, 4:8])
                nc.vector.tensor_add(
                    s2a_t[:, 0:4], s2a_t[:, 0:4], sb2_t[:, 8:12]
                )
                nc.vector.tensor_add(
                    s2a_t[:, 0:4], s2a_t[:, 0:4], sb2_t[:, 12:16]
                )
                nc.vector.tensor_add(s2a_t[:, 0:2], s2a_t[:, 0:2], s2a_t[:, 2:4])
                nc.vector.tensor_add(
                    ssq_t[:, None, :], s2a_t[:, 0:1], s2a_t[:, 1:2]
                )

                # ---- p = sum_d u * s ----
                nc.vector.tensor_mul(
                    t_t[:], u_t[:],
                    sb_t[:, :, None, :].to_broadcast((128, D, NG, O)),
                )
                halving_tree(t_t, p_t)

                # ---- b += f(sq) * (p - usq),  sq = |s|^2 - 2p + usq ----
                nc.vector.scalar_tensor_tensor(
                    sq_t[:], p_t[:], -2.0, usq_t[:],
                    op0=mybir.AluOpType.mult, op1=mybir.AluOpType.add,
                )
                nc.vector.tensor_add(
                    sq_t[:], sq_t[:], ssq_t[:, None, :].to_broadcast((128, NG, O))
                )
                nc.vector.tensor_sub(g_t[:], p_t[:], usq_t[:])
                # f = sq/((1+sq)*sqrt(sq)) = exp(0.5*ln(sq) - ln(1+sq))
                nc.scalar.activation(tm_t[:], sq_t[:], mybir.ActivationFunctionType.Ln)
                nc.scalar.activation(
                    tn_t[:], sq_t[:], mybir.ActivationFunctionType.Ln, bias=1.0
                )
                nc.vector.scalar_tensor_tensor(
                    tm_t[:], tm_t[:], 0.5, tn_t[:],
                    op0=mybir.AluOpType.mult, op1=mybir.AluOpType.subtract,
                )
                nc.scalar.activation(
                    tm_t[:], tm_t[:], mybir.ActivationFunctionType.Exp
                )
                nc.vector.tensor_mul(tm_t[:], tm_t[:], g_t[:])
                if it == 0:
                    nc.vector.tensor_copy(blog_t[:], tm_t[:])
                else:
                    nc.vector.tensor_add(blog_t[:], blog_t[:], tm_t[:])

    nc.compile()
    return nc


def _prep_x(x_core):
    # Block-diagonal stationary operand, built on the host:
    # xd[(il,k), bg, g, (il',b)] = x[bg*8+b, g*16+il, k] * (il == il')
    xr = x_core.reshape(NBG, GB, NG, IL, KD).transpose(4, 0, 2, 3, 1)  # k,bg,g,il,b
    xd = np.zeros((IL, KD, NBG, NG, IL, GB), np.float16)
    for il in range(IL):
        xd[il, :, :, :, il, :] = xr[:, :, :, il, :]
    return np.ascontiguousarray(xd.reshape(128, NBG, NG, 128))


def _prep_w(W0):
    # wr[(il,k), g, (o,d)] = W[o, g*16+il, d, k]
    return np.ascontiguousarray(
        W0.reshape(O, NG, IL, D, KD).transpose(2, 4, 1, 0, 3).reshape(128, NG, OD)
    ).astype(np.float16)


def _ones8_np():
    o = np.zeros((128, GB), np.float16)
    o[np.arange(128), np.arange(128) % GB] = 1.0
    return o


def _e8_np():
    e = np.zeros((GB, 128), np.float16)
    e[np.arange(128) % GB, np.arange(128)] = 1.0
    return e


def _make_runner(nc):
    """Build a cached jitted 8-core executor for the module (mirrors
    bass2jax.run_bass_via_pjrt but reusable across calls)."""
    import jax
    from jax.experimental.shard_map import shard_map
    from jax.sharding import Mesh, PartitionSpec

    from concourse import bass2jax as b2j

    b2j.install_neuronx_cc_hook()
    assert nc.dbg_addr is None
    partition_name = nc.partition_id_tensor.name if nc.partition_id_tensor else None

    in_names, out_names, out_avals = [], [], []
    for alloc in nc.m.functions[0].allocations:
        if not isinstance(alloc, mybir.MemoryLocationSet):
            continue
        name = alloc.memorylocations[0].name
        if alloc.kind == "ExternalInput":
            if name != partition_name:
                in_names.append(name)
        elif alloc.kind == "ExternalOutput":
            out_names.append(name)
            out_avals.append(
                jax.core.ShapedArray(
                    tuple(alloc.tensor_shape), mybir.dt.np(alloc.dtype)
                )
            )
    n_params = len(in_names)
    n_outs = len(out_names)
    all_names = in_names + out_names
    if partition_name is not None:
        all_names = all_names + [partition_name]
    donate = tuple(range(n_params, n_params + n_outs))

    def _body(*args):
        operands = list(args)
        if partition_name is not None:
            operands.append(b2j.partition_id_tensor())
        return tuple(
            b2j._bass_exec_p.bind(
                *operands,
                out_avals=tuple(out_avals),
                in_names=tuple(all_names),
                out_names=tuple(out_names),
                lowering_input_output_aliases=(),
                sim_require_finite=True,
                sim_require_nnan=True,
                nc=nc,
            )
        )

    devices = jax.devices()[:N_CORES]
    mesh = Mesh(np.asarray(devices), ("core",))
    in_specs = (PartitionSpec("core"),) * (n_params + n_outs)
    out_specs = (PartitionSpec("core"),) * n_outs
    sharded = jax.jit(
        shard_map(
            _body, mesh=mesh, in_specs=in_specs, out_specs=out_specs, check_rep=False
        ),
        donate_argnums=donate,
        keep_unused=True,
    )

    from jax.sharding import NamedSharding

    def prepare(in_maps):
        concat_in = [
            np.concatenate([np.asarray(m[name]) for m in in_maps], axis=0)
            for name in in_names
        ]
        sh = NamedSharding(mesh, PartitionSpec("core"))
        return [jax.device_put(a, sh) for a in concat_in]

    def run_prepared(dev_in, block=True):
        zeros = [
            np.zeros((N_CORES * a.shape[0],) + a.shape[1:], a.dtype)
            for a in out_avals
        ]
        outs = sharded(*dev_in, *zeros)
        if block:
            jax.block_until_ready(outs)
        return outs

    def run(in_maps):
        outs = [np.asarray(o) for o in run_prepared(prepare(in_maps))]
        return dict(zip(out_names, outs))

    run.prepare = prepare
    run.run_prepared = run_prepared
    return run


_RUNNERS = {}


def _get_runner(repeat=1):
    if repeat not in _RUNNERS:
        _RUNNERS[repeat] = _make_runner(_build_module(repeat=repeat))
    return _RUNNERS[repeat]


def _in_maps(x, W0):
    wr = _prep_w(W0)
    ones8 = _ones8_np()
    e8 = _e8_np()
    return [
        {"xd": _prep_x(x[c * BL : (c + 1) * BL]), "wr": wr, "ones8": ones8, "e8": e8}
        for c in range(N_CORES)
    ]


def kernel(x, y, W):
    x = np.asarray(x, dtype=np.float32)
    W0 = np.asarray(W, dtype=np.float32)[0]
    run = _get_runner()
    out = run(_in_maps(x, W0))["out"]
    return out.reshape(N_CORES * BL, O, D)


# revision 40
# speedup vs baseline: 1.4140x; 1.4140x over previous
"""DigitCaps dynamic-routing kernel for Trainium2 (8 NeuronCores, batch-sharded).

Full-input contract: kernel(x, y, W) -> (256, 10, 16) fp32.
  x: (256, 1152, 8) fp32, y: (256, 10) fp32 (unused by the reference), W: (1, 10, 1152, 16, 8) fp32.

Strategy (per core, 32 samples):
  - u_hat = einsum('oidk,bik->boid') computed on the PE array as 288 matmuls with a
    block-diagonal x operand: contraction dim = (i_local16, k8) = 128, output
    partitions = (i_local16, b8) = 128, moving dim = (o,d) = 160 per i-group.
  - u_hat kept resident in SBUF as fp16 [(il,b)=128, g=72, o=10, d=16]; the routing
    loop never touches HBM.
  - Routing iteration: t = c*u (DVE fp16 2x) -> s = col-sum via ones-block matmul
    (PE, PSUM accumulate over i-groups) -> broadcast s across partitions (DMA)
    -> p = sum_d u*s via DVE pairwise halving tree -> b-logit update from the
    closed form  b += f(sq)*(p - usq), sq = |s|^2 - 2p + usq, f = squash scale.
  - Final pass: s3 accumulated fp32 in PSUM, squash applied on fp32.
"""

import sys
from contextlib import ExitStack

sys.path.insert(0, "/opt/trn_rl_repo")

import functools

import numpy as np

from concourse import bacc, mybir, tile
from concourse import hw_specs as _hw_specs
from concourse.bass_utils import run_bass_kernel_spmd

# All ACT functions this kernel uses (Exp, Ln, Square, Copy, Identity) live
# together in the "natural_log_exp_and_others" table set, but the default
# placement pass maps Exp->set 0 and Ln->set 5, thrashing ~2.7us table loads
# between them on every routing iteration. Restrict those functions to the
# combined set (set ids stay positional, so other sets keep their ids).
_orig_get_activation_tables = _hw_specs.get_activation_tables


@functools.cache
def _patched_activation_tables(module_arch):
    tables = dict(_orig_get_activation_tables(module_arch))
    shared = None
    for name, funcs in tables.items():
        if name == "natural_log_exp_and_others":
            shared = funcs
    if shared is None:
        return tables
    strip = {
        f
        for f in (
            getattr(mybir.ActivationFunctionType, n, None)
            for n in ("Exp", "Ln", "Square", "Copy", "Identity")
        )
        if f is not None and f in shared
    }
    return {
        name: (funcs if name == "natural_log_exp_and_others" else funcs - strip)
        for name, funcs in tables.items()
    }


_hw_specs.get_activation_tables = _patched_activation_tables
bacc.get_activation_tables = _patched_activation_tables

F16 = mybir.dt.float16
F32 = mybir.dt.float32

N_CORES = 8
BL = 32          # batch per core
NG = 72          # i-groups (1152 / 16)
IL = 16          # i's per group
KD = 8           # in_dim
O = 10           # out_caps
D = 16           # out_dim
OD = O * D       # 160
NBG = 4          # sample-groups of 8 per core
GB = 8           # samples per group
EPS = 1e-8

_NC = None


def _build_module(repeat=1):
    nc = bacc.Bacc("TRN2", target_bir_lowering=False, debug=False)

    xd_d = nc.dram_tensor("xd", [128, NBG, NG, 128], F16, kind="ExternalInput")
    w_d = nc.dram_tensor("wr", [128, NG, OD], F16, kind="ExternalInput")
    ones_d = nc.dram_tensor("ones8", [128, GB], F16, kind="ExternalInput")
    e8_d = nc.dram_tensor("e8", [GB, 128], F16, kind="ExternalInput")
    out_d = nc.dram_tensor("out", [BL, O, D], F32, kind="ExternalOutput")

    with tile.TileContext(nc) as tc, ExitStack() as ctx:
        consts = ctx.enter_context(tc.tile_pool(name="consts", bufs=1))
        upool = ctx.enter_context(tc.tile_pool(name="u", bufs=2))
        tpool = ctx.enter_context(tc.tile_pool(name="t", bufs=2))
        spool = ctx.enter_context(tc.tile_pool(name="smalls", bufs=2))
        scr = ctx.enter_context(tc.tile_pool(name="scr", bufs=2))
        scr1 = ctx.enter_context(tc.tile_pool(name="scr1", bufs=2))
        lhsp = ctx.enter_context(tc.tile_pool(name="lhsp", bufs=2))
        psum = ctx.enter_context(tc.tile_pool(name="psum", bufs=4, space="PSUM"))
        psum_s = ctx.enter_context(tc.tile_pool(name="psum_s", bufs=2, space="PSUM"))
        psum_bc = ctx.enter_context(tc.tile_pool(name="psum_bc", bufs=2, space="PSUM"))

        w_tq = []
        for q in range(4):
            wq = consts.tile([128, 18, OD], F16, tag=f"w{q}")
            nc.sync.dma_start(wq[:], w_d[:, q * 18 : q * 18 + 18, :])
            w_tq.append(wq)
        ones_t = consts.tile([128, GB], F16)
        nc.sync.dma_start(ones_t[:], ones_d[:, :])
        e8_t = consts.tile([GB, 128], F16)
        nc.sync.dma_start(e8_t[:], e8_d[:, :])

        def make_stages(bg):
            # ---- per-sample-group tiles (pools are double-buffered, so two
            # groups can be live at once) ----
            u_t = upool.tile([128, D, NG, O], F16, tag="u")
            t_t = tpool.tile([128, D, NG, O], F16, tag="t")
            a_t = scr.tile([128, 8, NG, O], F16, tag="a")
            b4_t = scr1.tile([128, 4, NG, O], F16, tag="b4")
            c2_t = scr1.tile([128, 2, NG, O], F16, tag="c2")
            sb2_t = scr1.tile([128, D, O], F16, tag="sb2")

            p_t = spool.tile([128, NG, O], F16, tag="p")
            usq_t = spool.tile([128, NG, O], F16, tag="usq")
            blog_t = spool.tile([128, NG, O], F32, tag="blog")
            sq_t = spool.tile([128, NG, O], F16, tag="sq")
            g_t = spool.tile([128, NG, O], F16, tag="g")
            tm_t = spool.tile([128, NG, O], F16, tag="tm")
            tn_t = spool.tile([128, NG, O], F16, tag="tn")
            e_t = spool.tile([128, NG, O], F16, tag="e")
            c_t = spool.tile([128, NG, O], F16, tag="c")
            sig_t = spool.tile([128, NG], F32, tag="sig")
            sb_t = spool.tile([128, D, O], F16, tag="sb")
            ssq_t = spool.tile([128, O], F16, tag="ssq")
            s8f_t = spool.tile([8, D, O], F32, tag="s8f")
            s8h_t = spool.tile([8, D, O], F16, tag="s8h")
            ssq3_t = spool.tile([8, O], F32, tag="ssq3")
            f3a_t = spool.tile([8, O], F32, tag="f3a")
            f3b_t = spool.tile([8, O], F32, tag="f3b")
            v_t = spool.tile([8, D, O], F32, tag="v")
            vo_t = spool.tile([8, O, D], F32, tag="vo")

            def halving_tree(src, dst):
                # src: [128, 16, NG, O] fp16 -> dst: [128, NG, O] (sum over dim 1)
                nc.vector.tensor_add(a_t[:], src[:, 0:8], src[:, 8:16])
                nc.vector.tensor_add(b4_t[:], a_t[:, 0:4], a_t[:, 4:8])
                nc.vector.tensor_add(c2_t[:], b4_t[:, 0:2], b4_t[:, 2:4])
                nc.vector.tensor_add(dst[:, None, :, :], c2_t[:, 0:1], c2_t[:, 1:2])

            def stage_phase1():
                # u_hat for this sample group; (d, g, o)-ordered storage keeps
                # routing-loop broadcasts off the innermost axis (DVE 2x mode).
                for quarter in range(4):
                    g0 = quarter * 18
                    lhs_t = lhsp.tile([128, 18, 128], F16, tag="lhs")
                    nc.sync.dma_start(lhs_t[:], xd_d[:, bg, g0 : g0 + 18, :])
                    for m in range(6):  # triples of groups
                        pt = psum.tile([128, 3, O, D], F32, tag="pp")
                        for j in range(3):
                            g = g0 + m * 3 + j
                            nc.tensor.matmul(
                                pt[:, j],
                                lhsT=lhs_t[:, g - g0, :],
                                rhs=w_tq[quarter][:, g - g0, :],
                                start=True,
                                stop=True,
                            )
                        # transposing copy psum (g,o,d) -> u (d,g,o)
                        dst = u_t[:, :, g0 + m * 3 : g0 + m * 3 + 3, :]
                        nc.scalar.copy(dst.rearrange("p d g o -> p g o d"), pt[:])
                    # usq = sum_d u^2 on GPSIMD per quarter (overlaps phase 1);
                    # squares go through t_t, still free before iteration 0
                    gs = slice(g0, g0 + 18)
                    tq = t_t[:, :, gs, :]
                    nc.gpsimd.tensor_mul(tq, u_t[:, :, gs, :], u_t[:, :, gs, :])
                    nc.gpsimd.tensor_add(tq[:, 0:8], tq[:, 0:8], tq[:, 8:16])
                    nc.gpsimd.tensor_add(tq[:, 0:4], tq[:, 0:4], tq[:, 4:8])
                    nc.gpsimd.tensor_add(tq[:, 0:2], tq[:, 0:2], tq[:, 2:4])
                    nc.gpsimd.tensor_add(
                        usq_t[:, None, gs, :], tq[:, 0:1], tq[:, 1:2]
                    )

            def stage_iter(it):
                # ---- t = c * u ----
                if it == 0:
                    pass  # c is uniform 0.1: col-sum u directly, fold 0.1 into s
                else:
                    # softmax over o of b-logits (logits are tiny; skip max-sub)
                    nc.scalar.activation(
                        e_t[:], blog_t[:], mybir.ActivationFunctionType.Exp
                    )
                    nc.vector.tensor_reduce(
                        sig_t[:], e_t[:], axis=mybir.AxisListType.X,
                        op=mybir.AluOpType.add,
                    )
                    nc.vector.reciprocal_approx_fast(sig_t[:], sig_t[:])
                    nc.vector.tensor_mul(
                        c_t[:], e_t[:], sig_t[:, :, None].to_broadcast((128, NG, O))
                    )
                    nc.vector.tensor_mul(
                        t_t[:], u_t[:],
                        c_t[:, None, :, :].to_broadcast((128, D, NG, O)),
                    )

                # ---- s = sum_i t  (ones-block matmul, PSUM accumulate) ----
                src_t = u_t if it == 0 else t_t
                ps = psum_s.tile([8, D, 3, O], F32, tag="ps")
                for m in range(24):
                    nc.tensor.matmul(
                        ps[:],
                        lhsT=ones_t[:],
                        rhs=src_t[:, :, 3 * m : 3 * m + 3, :],
                        start=(m == 0),
                        stop=(m == 23),
                    )
                # fold the leftover g-triple axis with a strided reduce
                nc.vector.tensor_reduce(
                    s8f_t[:], ps[:].rearrange("b d three o -> b d o three"),
                    axis=mybir.AxisListType.X, op=mybir.AluOpType.add,
                )
                if it == 0:
                    nc.vector.tensor_scalar_mul(s8f_t[:], s8f_t[:], 0.1)

                if it == 2:
                    # ---- final squash(s) -> output (fp32) ----
                    nc.scalar.square(v_t[:], s8f_t[:])
                    nc.vector.tensor_reduce(
                        ssq3_t[:], v_t[:].rearrange("b d o -> b o d"),
                        axis=mybir.AxisListType.X, op=mybir.AluOpType.add,
                    )
                    nc.scalar.add(f3a_t[:], ssq3_t[:], 1.0)
                    nc.scalar.activation(
                        f3b_t[:], ssq3_t[:], mybir.ActivationFunctionType.Ln
                    )
                    nc.scalar.activation(
                        f3b_t[:], f3b_t[:], mybir.ActivationFunctionType.Exp, scale=0.5
                    )
                    nc.vector.scalar_tensor_tensor(
                        f3a_t[:], f3b_t[:], EPS, f3a_t[:],
                        op0=mybir.AluOpType.add, op1=mybir.AluOpType.mult,
                    )
                    nc.vector.reciprocal(f3a_t[:], f3a_t[:])
                    nc.vector.tensor_mul(f3a_t[:], f3a_t[:], ssq3_t[:])
                    nc.vector.tensor_mul(
                        v_t[:], s8f_t[:], f3a_t[:, None, :].to_broadcast((8, D, O))
                    )
                    nc.vector.tensor_copy(
                        vo_t[:].rearrange("b o d -> b d o"), v_t[:]
                    )
                    nc.sync.dma_start(out_d[bg * 8 : bg * 8 + 8], vo_t[:])
                    return

                # ---- broadcast s across partitions (delta-matmul) ----
                nc.vector.tensor_copy(s8h_t[:], s8f_t[:])
                ps_bc = psum_bc.tile([128, D, O], F32, tag="bc")
                nc.tensor.matmul(
                    ps_bc[:], lhsT=e8_t[:], rhs=s8h_t[:], start=True, stop=True
                )
                nc.vector.tensor_copy(sb_t[:], ps_bc[:])

                # ssq = sum_d s^2 (square on ACT, strided reduce over d)
                nc.scalar.square(sb2_t[:], sb_t[:])
                with nc.allow_low_precision(reason="16-term sum feeding b-logits"):
                    nc.vector.tensor_reduce(
                        ssq_t[:], sb2_t[:].rearrange("p d o -> p o d"),
                        axis=mybir.AxisListType.X, op=mybir.AluOpType.add,
                    )

                # ---- p = sum_d u * s ----
                nc.vector.tensor_mul(
                    t_t[:], u_t[:],
                    sb_t[:, :, None, :].to_broadcast((128, D, NG, O)),
                )
                halving_tree(t_t, p_t)

                # ---- b += f(sq) * (p - usq),  sq = |s|^2 - 2p + usq ----
                nc.vector.scalar_tensor_tensor(
                    sq_t[:], p_t[:], -2.0, usq_t[:],
                    op0=mybir.AluOpType.mult, op1=mybir.AluOpType.add,
                )
                nc.vector.tensor_add(
                    sq_t[:], sq_t[:], ssq_t[:, None, :].to_broadcast((128, NG, O))
                )
                nc.vector.tensor_sub(g_t[:], p_t[:], usq_t[:])
                # f = sq/((1+sq)*sqrt(sq)) = exp(0.5*ln(sq) - ln(1+sq))
                nc.scalar.activation(tm_t[:], sq_t[:], mybir.ActivationFunctionType.Ln)
                nc.scalar.activation(
                    tn_t[:], sq_t[:], mybir.ActivationFunctionType.Ln, bias=1.0
                )
                nc.vector.scalar_tensor_tensor(
                    tm_t[:], tm_t[:], 0.5, tn_t[:],
                    op0=mybir.AluOpType.mult, op1=mybir.AluOpType.subtract,
                )
                nc.scalar.activation(
                    tm_t[:], tm_t[:], mybir.ActivationFunctionType.Exp
                )
                nc.vector.tensor_mul(tm_t[:], tm_t[:], g_t[:])
                if it == 0:
                    nc.vector.tensor_copy(blog_t[:], tm_t[:])
                else:
                    nc.vector.tensor_add(blog_t[:], blog_t[:], tm_t[:])

            return [stage_phase1] + [
                (lambda it=it: stage_iter(it)) for it in range(3)
            ]

        # Interleave stage emission across pairs of sample groups so the Tile
        # scheduler can keep one group's DVE chain busy while the other's
        # PE/ACT/DMA work runs.
        for rep in range(repeat):
            for bg0 in range(0, NBG, 2):
                sa = make_stages(bg0)
                sb = make_stages(bg0 + 1)
                for fa, fb in zip(sa, sb):
                    fa()
                    fb()

    nc.compile()
    return nc


def _prep_x(x_core):
    # Block-diagonal stationary operand, built on the host:
    # xd[(il,k), bg, g, (il',b)] = x[bg*8+b, g*16+il, k] * (il == il')
    xr = x_core.reshape(NBG, GB, NG, IL, KD).transpose(4, 0, 2, 3, 1)  # k,bg,g,il,b
    xd = np.zeros((IL, KD, NBG, NG, IL, GB), np.float16)
    for il in range(IL):
        xd[il, :, :, :, il, :] = xr[:, :, :, il, :]
    return np.ascontiguousarray(xd.reshape(128, NBG, NG, 128))


def _prep_w(W0):
    # wr[(il,k), g, (o,d)] = W[o, g*16+il, d, k]
    return np.ascontiguousarray(
        W0.reshape(O, NG, IL, D, KD).transpose(2, 4, 1, 0, 3).reshape(128, NG, OD)
    ).astype(np.float16)


def _ones8_np():
    o = np.zeros((128, GB), np.float16)
    o[np.arange(128), np.arange(128) % GB] = 1.0
    return o


def _e8_np():
    e = np.zeros((GB, 128), np.float16)
    e[np.arange(128) % GB, np.arange(128)] = 1.0
    return e


def _make_runner(nc):
    """Build a cached jitted 8-core executor for the module (mirrors
    bass2jax.run_bass_via_pjrt but reusable across calls)."""
    import jax
    from jax.experimental.shard_map import shard_map
    from jax.sharding import Mesh, PartitionSpec

    from concourse import bass2jax as b2j

    b2j.install_neuronx_cc_hook()
    assert nc.dbg_addr is None
    partition_name = nc.partition_id_tensor.name if nc.partition_id_tensor else None

    in_names, out_names, out_avals = [], [], []
    for alloc in nc.m.functions[0].allocations:
        if not isinstance(alloc, mybir.MemoryLocationSet):
            continue
        name = alloc.memorylocations[0].name
        if alloc.kind == "ExternalInput":
            if name != partition_name:
                in_names.append(name)
        elif alloc.kind == "ExternalOutput":
            out_names.append(name)
            out_avals.append(
                jax.core.ShapedArray(
                    tuple(alloc.tensor_shape), mybir.dt.np(alloc.dtype)
                )
            )
    n_params = len(in_names)
    n_outs = len(out_names)
    all_names = in_names + out_names
    if partition_name is not None:
        all_names = all_names + [partition_name]
    donate = tuple(range(n_params, n_params + n_outs))

    def _body(*args):
        operands = list(args)
        if partition_name is not None:
            operands.append(b2j.partition_id_tensor())
        return tuple(
            b2j._bass_exec_p.bind(
                *operands,
                out_avals=tuple(out_avals),
                in_names=tuple(all_names),
                out_names=tuple(out_names),
                lowering_input_output_aliases=(),
                sim_require_finite=True,
                sim_require_nnan=True,
                nc=nc,
            )
        )

    devices = jax.devices()[:N_CORES]
    mesh = Mesh(np.asarray(devices), ("core",))
    in_specs = (PartitionSpec("core"),) * (n_params + n_outs)
    out_specs = (PartitionSpec("core"),) * n_outs
    sharded = jax.jit(
        shard_map(
            _body, mesh=mesh, in_specs=in_specs, out_specs=out_specs, check_rep=False
        ),
        donate_argnums=donate,
        keep_unused=True,
    )

    from jax.sharding import NamedSharding

    def prepare(in_maps):
        concat_in = [
            np.concatenate([np.asarray(m[name]) for m in in_maps], axis=0)
            for name in in_names
        ]
        sh = NamedSharding(mesh, PartitionSpec("core"))
        return [jax.device_put(a, sh) for a in concat_in]

    def run_prepared(dev_in, block=True):
        zeros = [
            np.zeros((N_CORES * a.shape[0],) + a.shape[1:], a.dtype)
            for a in out_avals
        ]
        outs = sharded(*dev_in, *zeros)
        if block:
            jax.block_until_ready(outs)
        return outs

    def run(in_maps):
        outs = [np.asarray(o) for o in run_prepared(prepare(in_maps))]
        return dict(zip(out_names, outs))

    run.prepare = prepare
    run.run_prepared = run_prepared
    return run


_RUNNERS = {}


def _get_runner(repeat=1):
    if repeat not in _RUNNERS:
        _RUNNERS[repeat] = _make_runner(_build_module(repeat=repeat))
    return _RUNNERS[repeat]


def _in_maps(x, W0):
    wr = _prep_w(W0)
    ones8 = _ones8_np()
    e8 = _e8_np()
    return [
        {"xd": _prep_x(x[c * BL : (c + 1) * BL]), "wr": wr, "ones8": ones8, "e8": e8}
        for c in range(N_CORES)
    ]


def kernel(x, y, W):
    x = np.asarray(x, dtype=np.float32)
    W0 = np.asarray(W, dtype=np.float32)[0]
    run = _get_runner()
    out = run(_in_maps(x, W0))["out"]
    return out.reshape(N_CORES * BL, O, D)
